# revision 17
# baseline (speedup 1.0000x reference)
"""DiT flow model forward pass on 8 Trainium2 NeuronCores.

Data-parallel over batch (8 batches/core, T=256 tokens/core), weights
replicated. Activations are kept transposed [D, T] on-chip so the whole layer
chain runs without activation transposes. Weights are pre-transposed to
K-major on the host.

Precision scheme: the activation stream is bf16 end-to-end (psim'd rel err
8.3e-3 vs the 2e-2 gate); PSUM accumulation and LN statistics are fp32.
The additive causal mask is preloaded into PSUM with an fp8-e5m2 DoubleRow
matmul (exact: mask values are 0/-28672), halving those matmul cycles.
The v-projection bias is folded into out_b on the host (softmax rows sum
to 1), removing the K=1 bias matmuls. LN produces a single bf16 output
(no duplicate fp32r/bf16 tile pair), and square/residual-add elementwise
work is offloaded to the otherwise-idle GpSimd engine off the critical path.

The hidden dimension is permuted per-head (even rotary slots first, odd
second) so RoPE becomes elementwise muls plus a contiguous 32-partition block
swap; the permutation is folded into the weights on the host.
"""

import sys

sys.path.insert(0, "/opt/trn_rl_repo")

from contextlib import ExitStack

import ml_dtypes
import numpy as np

import bass_rust
import concourse.bass as bass
import concourse.mybir as mybir
import concourse.tile as tile
from concourse.bass_utils import run_bass_kernel_spmd
from concourse.vector_clock import ScopedClock

B, S, LD, Hh, Ww = 64, 32, 16, 32, 18
D, NH, HD, FF, L = 512, 8, 64, 2048, 6
IN = LD * Hh * Ww
EPS = 1e-5
NCORES = 8
BSH = B // NCORES          # 8 batches per core
T = BSH * S                # 256 tokens per core
NEG = -28672.0             # additive mask value; e5m2-exact, exp() underflows to 0

f32 = mybir.dt.float32
f32r = mybir.dt.float32r
bf16 = mybir.dt.bfloat16
e5m2 = mybir.dt.float8e5
AT = mybir.ActivationFunctionType
ALU = mybir.AluOpType
DRM = mybir.MatmulPerfMode.DoubleRow

W_DT = bf16
IO_DT = bf16

# ---------------------------------------------------------------------------
# walrus in this container accepts at most ONE inline sync-wait per
# instruction; Tile can attach several. Split extras onto NoOp carriers.
# ---------------------------------------------------------------------------

def _patched_drain_and_barrier(self, tick_clock, wait_clock):
    nc = self.nc
    ticks = list(tick_clock.global_clock)
    for p, t in enumerate(ticks):
        if t > 0:
            vc = bass_rust.VectorClock([t if i == p else 0 for i in range(len(ticks))])
            nop_inst = nc.sync.nop(nofuse=True, hint=f"tailw{p}")
            wait_clock.add_sem_waits(nop_inst.ins, ScopedClock({None: vc}))
    nc.sync.drain()
    nc.all_engine_barrier()
    popped = nc._tile_sem_poison_stack.pop()
    assert popped is self._sem_poison
    nc.clear_and_free_semaphores(list(self.sems.allocated().values()))
    nc.all_engine_barrier()


def _split_multi_waits(nc, max_waits=1):
    for f in nc.m.functions:
        for blk in f.blocks:
            idx = 0
            while idx < len(blk.instructions):
                inst = blk.instructions[idx]
                si = inst.sync_info
                if si is not None and len(si.on_wait) > max_waits:
                    waits = list(si.on_wait)
                    for j, w in enumerate(waits[:-max_waits]):
                        carrier = mybir.InstNoOp(
                            name=f"{inst.name}_wsplit{j}",
                            engine=inst.engine,
                            bass_nofuse=True,
                            sync_info=mybir.SyncInfo(on_wait=[w], on_update=[]),
                        )
                        nc.register_instruction(carrier)
                        blk.instructions.insert(idx, carrier)
                        idx += 1
                    si.on_wait = waits[-max_waits:]
                idx += 1


tile.TileContext._drain_and_barrier = _patched_drain_and_barrier

# ---------------------------------------------------------------------------
# host-side numerics helpers
# ---------------------------------------------------------------------------

def _round_f32r(x):
    b = np.ascontiguousarray(x, dtype=np.float32).view(np.uint32)
    b = (b + np.uint32(0x7FF) + ((b >> np.uint32(12)) & np.uint32(1))) & np.uint32(0xFFFFF000)
    return b.view(np.float32)


def _cast(x, dt):
    if dt is bf16:
        return np.ascontiguousarray(np.asarray(x, np.float32)).astype(ml_dtypes.bfloat16)
    return _round_f32r(np.ascontiguousarray(x))


def _perm_src():
    p = np.empty(D, dtype=np.int64)
    for h in range(NH):
        for j in range(HD // 2):
            p[h * HD + j] = h * HD + 2 * j
            p[h * HD + HD // 2 + j] = h * HD + 2 * j + 1
    return p


# ---------------------------------------------------------------------------
# Bass kernel build
# ---------------------------------------------------------------------------

_CACHE = {}


def _build(nlayers):
    nc = bass.Bass()

    xT = nc.dram_tensor("xT", [IN, T], IO_DT, kind="ExternalInput")
    inp_wT = nc.dram_tensor("inp_wT", [IN, D], IO_DT, kind="ExternalInput")
    outp_wT = nc.dram_tensor("outp_wT", [D, IN], IO_DT, kind="ExternalInput")
    w_qkv = [nc.dram_tensor(f"w_qkv_{l}", [D, 3 * D], W_DT, kind="ExternalInput") for l in range(nlayers)]
    w_out = [nc.dram_tensor(f"w_out_{l}", [D, D], W_DT, kind="ExternalInput") for l in range(nlayers)]
    w_ff1 = [nc.dram_tensor(f"w_ff1_{l}", [D, FF], W_DT, kind="ExternalInput") for l in range(nlayers)]
    w_ff2 = [nc.dram_tensor(f"w_ff2_{l}", [FF, D], W_DT, kind="ExternalInput") for l in range(nlayers)]
    ct_d = nc.dram_tensor("ct", [D, T], bf16, kind="ExternalInput")
    sts_d = nc.dram_tensor("sts", [D, T], bf16, kind="ExternalInput")
    mask8_d = nc.dram_tensor("mask8", [128, 2, 128], e5m2, kind="ExternalInput")
    idw8_d = nc.dram_tensor("idw8", [128, 2, 128], e5m2, kind="ExternalInput")
    onesb_d = nc.dram_tensor("onesb", [128, 1], bf16, kind="ExternalInput")
    onesr_d = nc.dram_tensor("onesr", [1, 128], f32r, kind="ExternalInput")
    pswap_d = nc.dram_tensor("pswap", [128, 128], bf16, kind="ExternalInput")
    lnc_d = nc.dram_tensor("lnc", [D, 4 * nlayers], f32, kind="ExternalInput")
    qkb_d = nc.dram_tensor("qkb", [D, 2 * nlayers], f32, kind="ExternalInput")
    obt_d = nc.dram_tensor("obt", [D, nlayers], f32, kind="ExternalInput")
    ff1b_d = nc.dram_tensor("ff1bt", [FF, nlayers], f32, kind="ExternalInput")
    ff2b_d = nc.dram_tensor("ff2bt", [D, nlayers], f32, kind="ExternalInput")
    inpb_d = nc.dram_tensor("inpbt", [D, 1], f32, kind="ExternalInput")
    out_d = nc.dram_tensor("out", [T, IN], bf16, kind="ExternalOutput")

    with tile.TileContext(nc) as tc, ExitStack() as top:
        cp = top.enter_context(tc.tile_pool(name="consts", bufs=1))
        ap = top.enter_context(tc.tile_pool(name="acts", bufs=10))
        stp = top.enter_context(tc.tile_pool(name="stats", bufs=8))
        atp = top.enter_context(tc.tile_pool(name="attn", bufs=8))

        # ---- constants -----------------------------------------------------
        ct = cp.tile([128, 4, T], bf16, tag="ct")
        nc.sync.dma_start(ct[:], ct_d.rearrange("(kt p) t -> p kt t", p=128))
        sts = cp.tile([128, 4, T], bf16, tag="sts")
        nc.sync.dma_start(sts[:], sts_d.rearrange("(kt p) t -> p kt t", p=128))
        mask8 = cp.tile([128, 2, 128], e5m2, tag="mask8")
        nc.sync.dma_start(mask8[:], mask8_d[:])
        idw8 = cp.tile([128, 2, 128], e5m2, tag="idw8")
        nc.sync.dma_start(idw8[:], idw8_d[:])
        onesb = cp.tile([128, 1], bf16, tag="onesb")
        nc.sync.dma_start(onesb[:], onesb_d[:])
        onesr = cp.tile([1, 128], f32r, tag="onesr")
        nc.sync.dma_start(onesr[:], onesr_d[:])
        pswap = cp.tile([128, 128], bf16, tag="pswap")
        nc.sync.dma_start(pswap[:], pswap_d[:])
        lnc = cp.tile([128, 4, 4 * nlayers], f32, tag="lnc")
        nc.sync.dma_start(lnc[:], lnc_d.rearrange("(kt p) n -> p kt n", p=128))
        qkb = cp.tile([128, 4, 2 * nlayers], f32, tag="qkb")
        nc.sync.dma_start(qkb[:], qkb_d.rearrange("(kt p) n -> p kt n", p=128))
        obt = cp.tile([128, 4, nlayers], f32, tag="obt")
        nc.sync.dma_start(obt[:], obt_d.rearrange("(kt p) n -> p kt n", p=128))
        ff1b = cp.tile([128, 16, nlayers], f32, tag="ff1b")
        nc.sync.dma_start(ff1b[:], ff1b_d.rearrange("(kt p) n -> p kt n", p=128))
        ff2b = cp.tile([128, 4, nlayers], f32, tag="ff2b")
        nc.sync.dma_start(ff2b[:], ff2b_d.rearrange("(kt p) n -> p kt n", p=128))
        inpb = cp.tile([128, 4, 1], f32, tag="inpb")
        nc.sync.dma_start(inpb[:], inpb_d.rearrange("(kt p) n -> p kt n", p=128))
        epsc = cp.tile([1, 1], f32, tag="epsc")
        nc.vector.memset(epsc[:], float(D) * float(D) * EPS)

        hT = ap.tile([128, 4, T], bf16, tag="actb")

        # layer-phase pools opened first so layer-0 weights prefetch during
        # the input projection (stack allocator: inp pools nest inside)
        wp = top.enter_context(tc.tile_pool(name="wl", bufs=2))
        glp = top.enter_context(tc.tile_pool(name="gl", bufs=2))
        vp = top.enter_context(tc.tile_pool(name="vp", bufs=2))
        pmm = top.enter_context(tc.tile_pool(name="ps_mm", bufs=4, space="PSUM"))
        patt = top.enter_context(tc.tile_pool(name="ps_att", bufs=4, space="PSUM"))

        # ---- input projection: hT[D, T] = (x @ inp_w.T).T ------------------
        KT_IN = IN // 128          # 72 k-tiles
        CH = 9                     # k-tiles per streamed chunk
        with tc.tile_pool(name="inp_sb", bufs=2) as ip:
            hps = [pmm.tile([128, T], f32, tag="mm", name=f"hps{m}") for m in range(4)]
            for kc in range(KT_IN // CH):
                xc = ip.tile([128, CH, T], IO_DT, tag="xc")
                nc.sync.dma_start(
                    xc[:], xT[kc * CH * 128:(kc + 1) * CH * 128, :]
                    .rearrange("(kt p) t -> p kt t", p=128))
                wc = ip.tile([128, CH, D], IO_DT, tag="wc")
                nc.sync.dma_start(
                    wc[:], inp_wT[kc * CH * 128:(kc + 1) * CH * 128, :]
                    .rearrange("(kt p) n -> p kt n", p=128))
                for kk in range(CH):
                    first = kc == 0 and kk == 0
                    last = kc == KT_IN // CH - 1 and kk == CH - 1
                    for m in range(4):
                        nc.tensor.matmul(hps[m][:], wc[:, kk, m * 128:(m + 1) * 128],
                                         xc[:, kk, :], start=first, stop=last)
            for m in range(4):
                nc.scalar.activation(hT[:, m], hps[m][:], AT.Identity,
                                     bias=inpb[:, m, 0:1], scale=1.0)

        # ---- transformer layers -------------------------------------------
        if True:

            def ln_stats(src, m, sum_ps, sq_ps, sq):
                """Issue sum/sq-sum stat matmuls for k-tile m of bf16 src."""
                nc.tensor.matmul(sum_ps[:], onesb[:, 0:1], src[:, m],
                                 start=(m == 0), stop=(m == 3))
                nc.scalar.activation(sq[:, m], src[:, m], AT.Square)
                nc.tensor.matmul(sq_ps[:], onesb[:, 0:1], sq[:, m],
                                 start=(m == 0), stop=(m == 3))

            def ln_apply(src, wb_idx, dst, sum_ps, sq_ps):
                """Finalize stats and write normalized bf16 dst.

                rstd0 = 1/sqrt(D*S2 - S1^2 + D^2*eps) = 1/(D*sigma); the D
                factor is folded into the host-side LN scale table.
                """
                mu = stp.tile([1, T], f32, tag="st")
                nc.vector.tensor_scalar_mul(mu[:], sum_ps[:], 1.0 / D)
                s11 = stp.tile([1, T], f32, tag="st")
                nc.scalar.activation(s11[:], sum_ps[:], AT.Square)
                s2d = stp.tile([1, T], f32, tag="st")
                nc.vector.tensor_scalar_mul(s2d[:], sq_ps[:], float(D))
                c = stp.tile([1, T], f32, tag="st")
                nc.vector.tensor_tensor(c[:], s2d[:], s11[:], ALU.subtract)
                sd = stp.tile([1, T], f32, tag="st")
                nc.scalar.activation(sd[:], c[:], AT.Sqrt, bias=epsc[0:1, 0:1], scale=1.0)
                rm = stp.tile([1, 2, T], f32r, tag="st2")
                with nc.allow_low_precision(reason="rstd rounded to f32r for PE broadcast"):
                    nc.vector.reciprocal(rm[:, 0], sd[:])
                nc.vector.tensor_mul(rm[:, 1], mu[:], rm[:, 0].bitcast(f32))
                rmB = pmm.tile([128, 2, T], f32, tag="mm")
                nc.tensor.matmul(rmB[:], onesr[0:1, :], rm[:], start=True, stop=True)
                t0 = ap.tile([128, 4, T], f32, tag="lnsc", bufs=2)
                t1 = ap.tile([128, 4, T], f32, tag="lnsc", bufs=2)
                for m in range(4):
                    nc.vector.tensor_mul(t0[:, m], src[:, m], rmB[:, 0])
                    nc.vector.tensor_tensor(t1[:, m], t0[:, m], rmB[:, 1], ALU.subtract)
                    nc.scalar.activation(dst[:, m], t1[:, m], AT.Identity,
                                         bias=lnc[:, m, wb_idx + 1:wb_idx + 2],
                                         scale=lnc[:, m, wb_idx:wb_idx + 1])

            for l in range(nlayers):
                wqkv = wp.tile([128, 4, 3 * D], W_DT, tag="w")
                nc.sync.dma_start(wqkv[:], w_qkv[l].rearrange("(kt p) n -> p kt n", p=128))
                wout = wp.tile([128, 4, D], W_DT, tag="w")
                nc.sync.dma_start(wout[:], w_out[l].rearrange("(kt p) n -> p kt n", p=128))
                wff1 = wp.tile([128, 4, FF], W_DT, tag="w")
                nc.sync.dma_start(wff1[:], w_ff1[l].rearrange("(kt p) n -> p kt n", p=128))
                wff2 = wp.tile([128, 16, D], W_DT, tag="w")
                nc.sync.dma_start(wff2[:], w_ff2[l].rearrange("(kt p) n -> p kt n", p=128))

                # RoPE on hT -> hrT (pairs are (j, j+32) blocks within each head)
                hrT = ap.tile([128, 4, T], bf16, tag="actb")
                t2 = ap.tile([128, 4, T], f32, tag="ropesc", bufs=2)
                t1r = ap.tile([128, 4, T], f32, tag="ropesc", bufs=2)
                for m in range(4):
                    swp_ps = pmm.tile([128, T], f32, tag="mm")
                    nc.tensor.matmul(swp_ps[:], pswap[:], hT[:, m],
                                     start=True, stop=True)
                    nc.vector.tensor_mul(t2[:, m], hT[:, m], ct[:, m])
                    nc.vector.tensor_mul(t1r[:, m], swp_ps[:], sts[:, m])
                    nc.vector.tensor_tensor(hrT[:, m], t2[:, m], t1r[:, m], ALU.add)

                # q/k projections (Form T: out [Do,T])
                qT = ap.tile([128, 4, T], bf16, tag="actb")
                kT = ap.tile([128, 4, T], bf16, tag="actb")
                for qk, dst in ((0, qT), (1, kT)):
                    for m in range(4):
                        ps = pmm.tile([128, T], f32, tag="mm")
                        for k in range(4):
                            nc.tensor.matmul(
                                ps[:], wqkv[:, k, qk * D + m * 128: qk * D + (m + 1) * 128],
                                hrT[:, k], start=(k == 0), stop=(k == 3))
                        nc.scalar.activation(dst[:, m], ps[:], AT.Identity,
                                             bias=qkb[:, m, 2 * l + qk: 2 * l + qk + 1],
                                             scale=1.0)

                # v projection (Form N: out [T,D]); bias folded into out_b on host
                v = vp.tile([128, 2, D], W_DT, tag="v")
                for m2 in range(2):
                    for dh in range(2):
                        ps = pmm.tile([128, T], f32, tag="mm")
                        for k in range(4):
                            nc.tensor.matmul(
                                ps[:], hT[:, k, m2 * 128:(m2 + 1) * 128],
                                wqkv[:, k, 2 * D + dh * 256: 2 * D + (dh + 1) * 256],
                                start=(k == 0), stop=(k == 3))
                        nc.vector.tensor_copy(v[:, m2, dh * 256:(dh + 1) * 256], ps[:])

                # attention; sc phase of group kt overlaps ctx phase of kt-1
                ctxT = ap.tile([128, 4, T], bf16, tag="actb")
                atTs = {}

                def sc_phase(kt):
                    sc_t = {}
                    for half in range(2):
                        fr = slice(half * 128, (half + 1) * 128)
                        for hh in range(2):
                            sc = patt.tile([128, 128], f32, tag="sc",
                                           name=f"sc{kt}_{hh}_{half}")
                            nc.tensor.matmul(sc[:], mask8[:], idw8[:],
                                             start=True, stop=False, perf_mode=DRM)
                            sc_t[(hh, half)] = sc
                        for hh in range(2):  # adjacent for row-group packing
                            pb = 64 * hh
                            nc.tensor.matmul(sc_t[(hh, half)][:],
                                             qT[pb:pb + 64, kt, fr],
                                             kT[pb:pb + 64, kt, fr],
                                             start=False, stop=True)
                    for hh in range(2):
                        attn = atp.tile([128, 256], bf16, tag="atb")
                        for half in range(2):
                            att = atp.tile([128, 128], f32, tag="at")
                            rsum = stp.tile([128, 1], f32, tag="rs")
                            nc.scalar.activation(att[:], sc_t[(hh, half)][:], AT.Exp,
                                                 accum_out=rsum[:])
                            rinv = stp.tile([128, 1], f32, tag="rs")
                            nc.vector.reciprocal(rinv[:], rsum[:])
                            nc.vector.tensor_scalar_mul(attn[:, half * 128:(half + 1) * 128],
                                                        att[:], rinv[:])
                        atT = atp.tile([128, 256], bf16, tag="atb")
                        nc.vector.transpose(atT[:], attn[:])
                        atTs[(kt, hh)] = atT

                def ctx_phase(kt):
                    cps = pmm.tile([128, T], f32, tag="mm")
                    for half in range(2):
                        fr = slice(half * 128, (half + 1) * 128)
                        for hh in range(2):  # adjacent for col-group packing
                            h = 2 * kt + hh
                            pb = 64 * hh
                            nc.tensor.matmul(cps[pb:pb + 64, fr],
                                             v[:, half, h * 64:(h + 1) * 64],
                                             atTs[(kt, hh)][:, fr], start=True, stop=True)
                    nc.vector.tensor_copy(ctxT[:, kt, :], cps[:])

                for kt in range(4):
                    sc_phase(kt)
                    if kt >= 1:
                        ctx_phase(kt - 1)
                ctx_phase(3)

                # out projection + residual + ln1 stats inline
                h1pre = ap.tile([128, 4, T], bf16, tag="actb")
                sa4 = ap.tile([128, 4, T], f32, tag="resc", bufs=2)
                sum1 = pmm.tile([1, T], f32, tag="mm")
                sqs1 = pmm.tile([1, T], f32, tag="mm")
                sq1 = ap.tile([128, 4, T], bf16, tag="sqt", bufs=2)
                for m in range(4):
                    ps = pmm.tile([128, T], f32, tag="mm")
                    for k in range(4):
                        nc.tensor.matmul(ps[:], wout[:, k, m * 128:(m + 1) * 128],
                                         ctxT[:, k], start=(k == 0), stop=(k == 3))
                    nc.scalar.activation(sa4[:, m], ps[:], AT.Identity,
                                         bias=obt[:, m, l:l + 1], scale=1.0)
                    nc.vector.tensor_tensor(h1pre[:, m], sa4[:, m], hT[:, m], ALU.add)
                for m in range(4):
                    ln_stats(h1pre, m, sum1, sqs1, sq1)

                h1T = ap.tile([128, 4, T], bf16, tag="actb")
                ln_apply(h1pre, 4 * l, h1T, sum1, sqs1)

                # FFN
                gl = glp.tile([128, 16, T], W_DT, tag="gl")
                for ft in range(16):
                    ps = pmm.tile([128, T], f32, tag="mm")
                    for k in range(4):
                        nc.tensor.matmul(ps[:], wff1[:, k, ft * 128:(ft + 1) * 128],
                                         h1T[:, k], start=(k == 0), stop=(k == 3))
                    nc.scalar.activation(gl[:, ft], ps[:], AT.Gelu,
                                         bias=ff1b[:, ft, l:l + 1], scale=1.0)
                h2pre = ap.tile([128, 4, T], bf16, tag="actb")
                ff4 = ap.tile([128, 4, T], f32, tag="resc", bufs=2)
                sum2 = pmm.tile([1, T], f32, tag="mm")
                sqs2 = pmm.tile([1, T], f32, tag="mm")
                sq2 = ap.tile([128, 4, T], bf16, tag="sqt", bufs=2)
                for m in range(4):
                    ps = pmm.tile([128, T], f32, tag="mm")
                    for k in range(16):
                        nc.tensor.matmul(ps[:], wff2[:, k, m * 128:(m + 1) * 128],
                                         gl[:, k], start=(k == 0), stop=(k == 15))
                    nc.scalar.activation(ff4[:, m], ps[:], AT.Identity,
                                         bias=ff2b[:, m, l:l + 1], scale=1.0)
                    nc.vector.tensor_tensor(h2pre[:, m], ff4[:, m], h1T[:, m], ALU.add)
                for m in range(4):
                    ln_stats(h2pre, m, sum2, sqs2, sq2)

                hT = ap.tile([128, 4, T], bf16, tag="actb")
                ln_apply(h2pre, 4 * l + 2, hT, sum2, sqs2)

        # ---- output projection: out[T, IN] = h @ outp_w.T ------------------
        NCH = 9
        CW = IN // NCH            # 1024 columns per chunk
        with tc.tile_pool(name="op_sb", bufs=3) as op:
            for ncr in range(NCH):
                wc = op.tile([128, 4, CW], IO_DT, tag="wco")
                nc.sync.dma_start(
                    wc[:], outp_wT.rearrange("(kt p) n -> p kt n", p=128)
                    [:, :, ncr * CW:(ncr + 1) * CW])
                for m2 in range(2):
                    for nn in range(4):
                        ps = pmm.tile([128, T], f32, tag="mm")
                        for k in range(4):
                            nc.tensor.matmul(ps[:], hT[:, k, m2 * 128:(m2 + 1) * 128],
                                             wc[:, k, nn * 256:(nn + 1) * 256],
                                             start=(k == 0), stop=(k == 3))
                        osb = op.tile([128, 256], bf16, tag="osb")
                        nc.vector.tensor_copy(osb[:], ps[:])
                        nc.sync.dma_start(
                            out_d[m2 * 128:(m2 + 1) * 128,
                                  ncr * CW + nn * 256: ncr * CW + (nn + 1) * 256],
                            osb[:])

    _split_multi_waits(nc)
    return nc


# ---------------------------------------------------------------------------
# host wrapper
# ---------------------------------------------------------------------------

def _prepare(inputs, nlayers):
    perm = _perm_src()
    inp_w = np.asarray(inputs["inp_w"], np.float32)
    inp_b = np.asarray(inputs["inp_b"], np.float32)
    in_proj_w = np.asarray(inputs["in_proj_w"], np.float32)
    in_proj_b = np.asarray(inputs["in_proj_b"], np.float32)
    out_w = np.asarray(inputs["out_w"], np.float32)
    out_b = np.asarray(inputs["out_b"], np.float32)
    ln1_w = np.asarray(inputs["ln1_w"], np.float32)
    ln1_b = np.asarray(inputs["ln1_b"], np.float32)
    ln2_w = np.asarray(inputs["ln2_w"], np.float32)
    ln2_b = np.asarray(inputs["ln2_b"], np.float32)
    ff1_w = np.asarray(inputs["ff1_w"], np.float32)
    ff1_b = np.asarray(inputs["ff1_b"], np.float32)
    ff2_w = np.asarray(inputs["ff2_w"], np.float32)
    ff2_b = np.asarray(inputs["ff2_b"], np.float32)
    outp_w = np.asarray(inputs["outp_w"], np.float32)

    common = {}
    common["inp_wT"] = _cast(inp_w[perm, :].T, IO_DT)
    common["outp_wT"] = _cast(outp_w[:, perm].T, IO_DT)
    qkb = np.zeros((D, 2 * nlayers), np.float32)
    obt = np.zeros((D, nlayers), np.float32)
    ff1bt = np.zeros((FF, nlayers), np.float32)
    ff2bt = np.zeros((D, nlayers), np.float32)
    lnc = np.zeros((D, 4 * nlayers), np.float32)
    for l in range(nlayers):
        wq = in_proj_w[l, :D][perm][:, perm] / np.sqrt(HD)
        wk = in_proj_w[l, D:2 * D][perm][:, perm]
        wv = in_proj_w[l, 2 * D:][perm][:, perm]
        common[f"w_qkv_{l}"] = _cast(np.concatenate([wq.T, wk.T, wv.T], axis=1), W_DT)
        common[f"w_out_{l}"] = _cast(out_w[l][perm][:, perm].T, W_DT)
        common[f"w_ff1_{l}"] = _cast(ff1_w[l][:, perm].T, W_DT)
        common[f"w_ff2_{l}"] = _cast(ff2_w[l][perm, :].T, W_DT)
        qkb[:, 2 * l] = in_proj_b[l, :D][perm] / np.sqrt(HD)
        qkb[:, 2 * l + 1] = in_proj_b[l, D:2 * D][perm]
        # v bias folded through attention (softmax rows sum to 1): obt = out_b + out_w @ bv
        bv = in_proj_b[l, 2 * D:]
        obt[:, l] = (out_b[l] + out_w[l] @ bv)[perm]
        ff1bt[:, l] = ff1_b[l]
        ff2bt[:, l] = ff2_b[l][perm]
        lnc[:, 4 * l] = ln1_w[l][perm] * D
        lnc[:, 4 * l + 1] = ln1_b[l][perm]
        lnc[:, 4 * l + 2] = ln2_w[l][perm] * D
        lnc[:, 4 * l + 3] = ln2_b[l][perm]
    common["qkb"] = qkb
    common["obt"] = obt
    common["ff1bt"] = ff1bt
    common["ff2bt"] = ff2bt
    common["lnc"] = lnc
    common["inpbt"] = inp_b[perm].reshape(D, 1).astype(np.float32)

    # rope tables in permuted space: partition d = h*64 + j
    theta = 1.0 / (10000.0 ** (np.arange(0, HD, 2, dtype=np.float32) / HD))  # [32]
    pos = np.arange(S, dtype=np.float32)
    ang = pos[:, None] * theta[None, :]                  # [S, 32]
    cos_t = np.cos(ang).astype(np.float32)
    sin_t = np.sin(ang).astype(np.float32)
    ct = np.zeros((D, T), np.float32)
    sts = np.zeros((D, T), np.float32)
    s_of_t = np.tile(np.arange(S), BSH)                  # position of each token
    for h in range(NH):
        for j in range(HD // 2):
            ct[h * HD + j, :] = cos_t[s_of_t, j]
            ct[h * HD + HD // 2 + j, :] = cos_t[s_of_t, j]
            sts[h * HD + j, :] = -sin_t[s_of_t, j]
            sts[h * HD + HD // 2 + j, :] = sin_t[s_of_t, j]
    common["ct"] = _cast(ct, bf16)
    common["sts"] = _cast(sts, bf16)

    # additive causal+block mask as fp8-e5m2 DoubleRow pair (subtile 1 is zero)
    mask = np.full((128, 128), NEG, np.float32)
    for blk in range(4):
        for i in range(S):
            mask[blk * S + i, blk * S: blk * S + i + 1] = 0.0
    m8 = np.zeros((128, 2, 128), ml_dtypes.float8_e5m2)
    m8[:, 0, :] = mask.T.astype(ml_dtypes.float8_e5m2)
    common["mask8"] = m8
    i8 = np.zeros((128, 2, 128), ml_dtypes.float8_e5m2)
    i8[:, 0, :] = np.eye(128, dtype=np.float32).astype(ml_dtypes.float8_e5m2)
    common["idw8"] = i8
    common["onesb"] = _cast(np.ones((128, 1), np.float32), bf16)
    common["onesr"] = _round_f32r(np.ones((1, 128), np.float32))
    psw = np.zeros((128, 128), np.float32)
    for h2 in range(2):
        b0 = 64 * h2
        for j in range(32):
            psw[b0 + 32 + j, b0 + j] = 1.0      # lhsT[k, m]: out[m] sums in[k]
            psw[b0 + j, b0 + 32 + j] = 1.0
    common["pswap"] = _cast(psw, bf16)
    return common


def kernel(**inputs):
    nlayers = _CACHE.setdefault("nlayers", L)
    x = np.asarray(inputs["x"], np.float32)
    if "bass" not in _CACHE:
        _CACHE["bass"] = _build(nlayers)
    nc = _CACHE["bass"]
    common = _prepare(inputs, nlayers)
    in_maps = []
    for c in range(NCORES):
        m = dict(common)
        xc = x[c * BSH:(c + 1) * BSH].reshape(T, IN)
        m["xT"] = _cast(xc.T, IO_DT)
        in_maps.append(m)
    res = run_bass_kernel_spmd(nc, in_maps, core_ids=list(range(NCORES)))
    _CACHE["res"] = res
    outp_b = np.asarray(inputs["outp_b"], np.float32)
    outs = [np.asarray(res.results[c]["out"], np.float32) + outp_b[None, :]
            for c in range(NCORES)]
    full = np.concatenate(outs, axis=0).reshape(B, S, LD, Hh, Ww)
    return full.astype(np.float32)


# revision 19
# speedup vs baseline: 1.0881x; 1.0881x over previous
"""DiT flow model forward pass on 8 Trainium2 NeuronCores.

Data-parallel over batch (8 batches/core, T=256 tokens/core), weights
replicated. Activations are kept transposed [D, T] on-chip so the whole layer
chain runs without activation transposes. Weights are pre-transposed to
K-major on the host.

Precision scheme: the activation stream is bf16 end-to-end (psim'd rel err
8.3e-3 vs the 2e-2 gate); PSUM accumulation and LN statistics are fp32.
The additive causal mask is preloaded into PSUM with an fp8-e5m2 DoubleRow
matmul (exact: mask values are 0/-28672), halving those matmul cycles.
The v-projection bias is folded into out_b on the host (softmax rows sum
to 1), removing the K=1 bias matmuls. LN produces a single bf16 output
(no duplicate fp32r/bf16 tile pair) and finalizes its statistics with a
shortened dependency chain (rstd computed from raw S1/S2 sums, the 1/D
factor folded into the host-side scale table). Attention score/context
matmuls for the two heads of a pair are issued adjacently so their
64-partition tiles land on distinct PE row/col groups and run packed;
the context phase of head-group kt is issued after the score matmuls of
kt+1 so the PE stays busy across each softmax chain.

The hidden dimension is permuted per-head (even rotary slots first, odd
second) so RoPE becomes elementwise muls plus a contiguous 32-partition block
swap; the permutation is folded into the weights on the host.
"""

import sys

sys.path.insert(0, "/opt/trn_rl_repo")

from contextlib import ExitStack

import ml_dtypes
import numpy as np

import bass_rust
import concourse.bass as bass
import concourse.mybir as mybir
import concourse.tile as tile
from concourse.bass_utils import run_bass_kernel_spmd
from concourse.vector_clock import ScopedClock

B, S, LD, Hh, Ww = 64, 32, 16, 32, 18
D, NH, HD, FF, L = 512, 8, 64, 2048, 6
IN = LD * Hh * Ww
EPS = 1e-5
NCORES = 8
BSH = B // NCORES          # 8 batches per core
T = BSH * S                # 256 tokens per core
NEG = -28672.0             # additive mask value; e5m2-exact, exp() underflows to 0

f32 = mybir.dt.float32
f32r = mybir.dt.float32r
bf16 = mybir.dt.bfloat16
e5m2 = mybir.dt.float8e5
AT = mybir.ActivationFunctionType
ALU = mybir.AluOpType
DRM = mybir.MatmulPerfMode.DoubleRow

W_DT = bf16
IO_DT = bf16

# ---------------------------------------------------------------------------
# walrus in this container accepts at most ONE inline sync-wait per
# instruction; Tile can attach several. Split extras onto NoOp carriers.
# ---------------------------------------------------------------------------

def _patched_drain_and_barrier(self, tick_clock, wait_clock):
    nc = self.nc
    ticks = list(tick_clock.global_clock)
    for p, t in enumerate(ticks):
        if t > 0:
            vc = bass_rust.VectorClock([t if i == p else 0 for i in range(len(ticks))])
            nop_inst = nc.sync.nop(nofuse=True, hint=f"tailw{p}")
            wait_clock.add_sem_waits(nop_inst.ins, ScopedClock({None: vc}))
    nc.sync.drain()
    nc.all_engine_barrier()
    popped = nc._tile_sem_poison_stack.pop()
    assert popped is self._sem_poison
    nc.clear_and_free_semaphores(list(self.sems.allocated().values()))
    nc.all_engine_barrier()


def _split_multi_waits(nc, max_waits=1):
    for f in nc.m.functions:
        for blk in f.blocks:
            idx = 0
            while idx < len(blk.instructions):
                inst = blk.instructions[idx]
                si = inst.sync_info
                if si is not None and len(si.on_wait) > max_waits:
                    waits = list(si.on_wait)
                    for j, w in enumerate(waits[:-max_waits]):
                        carrier = mybir.InstNoOp(
                            name=f"{inst.name}_wsplit{j}",
                            engine=inst.engine,
                            bass_nofuse=True,
                            sync_info=mybir.SyncInfo(on_wait=[w], on_update=[]),
                        )
                        nc.register_instruction(carrier)
                        blk.instructions.insert(idx, carrier)
                        idx += 1
                    si.on_wait = waits[-max_waits:]
                idx += 1


tile.TileContext._drain_and_barrier = _patched_drain_and_barrier

# ---------------------------------------------------------------------------
# host-side numerics helpers
# ---------------------------------------------------------------------------

def _round_f32r(x):
    b = np.ascontiguousarray(x, dtype=np.float32).view(np.uint32)
    b = (b + np.uint32(0x7FF) + ((b >> np.uint32(12)) & np.uint32(1))) & np.uint32(0xFFFFF000)
    return b.view(np.float32)


def _cast(x, dt):
    if dt is bf16:
        return np.ascontiguousarray(np.asarray(x, np.float32)).astype(ml_dtypes.bfloat16)
    return _round_f32r(np.ascontiguousarray(x))


def _perm_src():
    p = np.empty(D, dtype=np.int64)
    for h in range(NH):
        for j in range(HD // 2):
            p[h * HD + j] = h * HD + 2 * j
            p[h * HD + HD // 2 + j] = h * HD + 2 * j + 1
    return p


# ---------------------------------------------------------------------------
# Bass kernel build
# ---------------------------------------------------------------------------

_CACHE = {}


def _build(nlayers):
    nc = bass.Bass()

    xT = nc.dram_tensor("xT", [IN, T], IO_DT, kind="ExternalInput")
    inp_wT = nc.dram_tensor("inp_wT", [IN, D], IO_DT, kind="ExternalInput")
    outp_wT = nc.dram_tensor("outp_wT", [D, IN], IO_DT, kind="ExternalInput")
    w_qkv = [nc.dram_tensor(f"w_qkv_{l}", [D, 3 * D], W_DT, kind="ExternalInput") for l in range(nlayers)]
    w_out = [nc.dram_tensor(f"w_out_{l}", [D, D], W_DT, kind="ExternalInput") for l in range(nlayers)]
    w_ff1 = [nc.dram_tensor(f"w_ff1_{l}", [D, FF], W_DT, kind="ExternalInput") for l in range(nlayers)]
    w_ff2 = [nc.dram_tensor(f"w_ff2_{l}", [FF, D], W_DT, kind="ExternalInput") for l in range(nlayers)]
    ct_d = nc.dram_tensor("ct", [D, T], bf16, kind="ExternalInput")
    sts_d = nc.dram_tensor("sts", [D, T], bf16, kind="ExternalInput")
    mask8_d = nc.dram_tensor("mask8", [128, 2, 128], e5m2, kind="ExternalInput")
    idw8_d = nc.dram_tensor("idw8", [128, 2, 128], e5m2, kind="ExternalInput")
    onesb_d = nc.dram_tensor("onesb", [128, 1], bf16, kind="ExternalInput")
    onesr_d = nc.dram_tensor("onesr", [1, 128], f32r, kind="ExternalInput")
    pswap_d = nc.dram_tensor("pswap", [128, 128], bf16, kind="ExternalInput")
    lnc_d = nc.dram_tensor("lnc", [D, 4 * nlayers], f32, kind="ExternalInput")
    qkb_d = nc.dram_tensor("qkb", [D, 2 * nlayers], f32, kind="ExternalInput")
    obt_d = nc.dram_tensor("obt", [D, nlayers], f32, kind="ExternalInput")
    ff1b_d = nc.dram_tensor("ff1bt", [FF, nlayers], f32, kind="ExternalInput")
    ff2b_d = nc.dram_tensor("ff2bt", [D, nlayers], f32, kind="ExternalInput")
    inpb_d = nc.dram_tensor("inpbt", [D, 1], f32, kind="ExternalInput")
    out_d = nc.dram_tensor("out", [T, IN], bf16, kind="ExternalOutput")

    with tile.TileContext(nc) as tc, ExitStack() as top:
        cp = top.enter_context(tc.tile_pool(name="consts", bufs=1))
        ap = top.enter_context(tc.tile_pool(name="acts", bufs=10))
        stp = top.enter_context(tc.tile_pool(name="stats", bufs=8))
        atp = top.enter_context(tc.tile_pool(name="attn", bufs=8))

        # ---- constants -----------------------------------------------------
        ct = cp.tile([128, 4, T], bf16, tag="ct")
        nc.sync.dma_start(ct[:], ct_d.rearrange("(kt p) t -> p kt t", p=128))
        sts = cp.tile([128, 4, T], bf16, tag="sts")
        nc.sync.dma_start(sts[:], sts_d.rearrange("(kt p) t -> p kt t", p=128))
        mask8 = cp.tile([128, 2, 128], e5m2, tag="mask8")
        nc.sync.dma_start(mask8[:], mask8_d[:])
        idw8 = cp.tile([128, 2, 128], e5m2, tag="idw8")
        nc.sync.dma_start(idw8[:], idw8_d[:])
        onesb = cp.tile([128, 1], bf16, tag="onesb")
        nc.sync.dma_start(onesb[:], onesb_d[:])
        onesr = cp.tile([1, 128], f32r, tag="onesr")
        nc.sync.dma_start(onesr[:], onesr_d[:])
        pswap = cp.tile([128, 128], bf16, tag="pswap")
        nc.sync.dma_start(pswap[:], pswap_d[:])
        lnc = cp.tile([128, 4, 4 * nlayers], f32, tag="lnc")
        nc.sync.dma_start(lnc[:], lnc_d.rearrange("(kt p) n -> p kt n", p=128))
        qkb = cp.tile([128, 4, 2 * nlayers], f32, tag="qkb")
        nc.sync.dma_start(qkb[:], qkb_d.rearrange("(kt p) n -> p kt n", p=128))
        obt = cp.tile([128, 4, nlayers], f32, tag="obt")
        nc.sync.dma_start(obt[:], obt_d.rearrange("(kt p) n -> p kt n", p=128))
        ff1b = cp.tile([128, 16, nlayers], f32, tag="ff1b")
        nc.sync.dma_start(ff1b[:], ff1b_d.rearrange("(kt p) n -> p kt n", p=128))
        ff2b = cp.tile([128, 4, nlayers], f32, tag="ff2b")
        nc.sync.dma_start(ff2b[:], ff2b_d.rearrange("(kt p) n -> p kt n", p=128))
        inpb = cp.tile([128, 4, 1], f32, tag="inpb")
        nc.sync.dma_start(inpb[:], inpb_d.rearrange("(kt p) n -> p kt n", p=128))
        epsc = cp.tile([1, 1], f32, tag="epsc")
        nc.vector.memset(epsc[:], float(D) * float(D) * EPS)

        hT = ap.tile([128, 4, T], bf16, tag="actb")

        # layer-phase pools opened first so layer-0 weights prefetch during
        # the input projection (stack allocator: inp pools nest inside)
        wp = top.enter_context(tc.tile_pool(name="wl", bufs=2))
        glp = top.enter_context(tc.tile_pool(name="gl", bufs=2))
        vp = top.enter_context(tc.tile_pool(name="vp", bufs=2))
        pmm = top.enter_context(tc.tile_pool(name="ps_mm", bufs=4, space="PSUM"))
        patt = top.enter_context(tc.tile_pool(name="ps_att", bufs=4, space="PSUM"))

        # ---- input projection: hT[D, T] = (x @ inp_w.T).T ------------------
        KT_IN = IN // 128          # 72 k-tiles
        CH = 6                     # k-tiles per streamed chunk
        with tc.tile_pool(name="inp_sb", bufs=3) as ip:
            hps = [pmm.tile([128, T], f32, tag="mm", name=f"hps{m}") for m in range(4)]
            for kc in range(KT_IN // CH):
                xc = ip.tile([128, CH, T], IO_DT, tag="xc")
                nc.sync.dma_start(
                    xc[:], xT[kc * CH * 128:(kc + 1) * CH * 128, :]
                    .rearrange("(kt p) t -> p kt t", p=128))
                wc = ip.tile([128, CH, D], IO_DT, tag="wc")
                nc.sync.dma_start(
                    wc[:], inp_wT[kc * CH * 128:(kc + 1) * CH * 128, :]
                    .rearrange("(kt p) n -> p kt n", p=128))
                for kk in range(CH):
                    first = kc == 0 and kk == 0
                    last = kc == KT_IN // CH - 1 and kk == CH - 1
                    for m in range(4):
                        nc.tensor.matmul(hps[m][:], wc[:, kk, m * 128:(m + 1) * 128],
                                         xc[:, kk, :], start=first, stop=last)
            for m in range(4):
                nc.scalar.activation(hT[:, m], hps[m][:], AT.Identity,
                                     bias=inpb[:, m, 0:1], scale=1.0)

        # ---- transformer layers -------------------------------------------
        if True:

            def ln_stats(src, m, sum_ps, sq_ps, sq):
                """Issue sum/sq-sum stat matmuls for k-tile m of bf16 src."""
                nc.tensor.matmul(sum_ps[:], onesb[:, 0:1], src[:, m],
                                 start=(m == 0), stop=(m == 3))
                nc.scalar.activation(sq[:, m], src[:, m], AT.Square)
                nc.tensor.matmul(sq_ps[:], onesb[:, 0:1], sq[:, m],
                                 start=(m == 0), stop=(m == 3))

            def ln_apply(src, wb_idx, dst, sum_ps, sq_ps):
                """Finalize stats and write normalized bf16 dst.

                rstd0 = 1/sqrt(D*S2 - S1^2 + D^2*eps) = 1/(D*sigma); the D
                factor is folded into the host-side LN scale table.
                """
                mu = stp.tile([1, T], f32, tag="st")
                nc.vector.tensor_scalar_mul(mu[:], sum_ps[:], 1.0 / D)
                s11 = stp.tile([1, T], f32, tag="st")
                nc.scalar.activation(s11[:], sum_ps[:], AT.Square)
                s2d = stp.tile([1, T], f32, tag="st")
                nc.vector.tensor_scalar_mul(s2d[:], sq_ps[:], float(D))
                c = stp.tile([1, T], f32, tag="st")
                nc.vector.tensor_tensor(c[:], s2d[:], s11[:], ALU.subtract)
                sd = stp.tile([1, T], f32, tag="st")
                nc.scalar.activation(sd[:], c[:], AT.Sqrt, bias=epsc[0:1, 0:1], scale=1.0)
                rm = stp.tile([1, 2, T], f32r, tag="st2")
                with nc.allow_low_precision(reason="rstd rounded to f32r for PE broadcast"):
                    nc.vector.reciprocal(rm[:, 0], sd[:])
                nc.vector.tensor_mul(rm[:, 1], mu[:], rm[:, 0].bitcast(f32))
                rmB = pmm.tile([128, 2, T], f32, tag="mm")
                nc.tensor.matmul(rmB[:], onesr[0:1, :], rm[:], start=True, stop=True)
                t0 = ap.tile([128, 4, T], f32, tag="lnsc", bufs=2)
                t1 = ap.tile([128, 4, T], f32, tag="lnsc", bufs=2)
                for m in range(4):
                    nc.vector.tensor_mul(t0[:, m], src[:, m], rmB[:, 0])
                    nc.vector.tensor_tensor(t1[:, m], t0[:, m], rmB[:, 1], ALU.subtract)
                    nc.scalar.activation(dst[:, m], t1[:, m], AT.Identity,
                                         bias=lnc[:, m, wb_idx + 1:wb_idx + 2],
                                         scale=lnc[:, m, wb_idx:wb_idx + 1])

            for l in range(nlayers):
                wqkv = wp.tile([128, 4, 3 * D], W_DT, tag="w")
                nc.sync.dma_start(wqkv[:], w_qkv[l].rearrange("(kt p) n -> p kt n", p=128))
                wout = wp.tile([128, 4, D], W_DT, tag="w")
                nc.sync.dma_start(wout[:], w_out[l].rearrange("(kt p) n -> p kt n", p=128))
                wff1 = wp.tile([128, 4, FF], W_DT, tag="w")
                nc.sync.dma_start(wff1[:], w_ff1[l].rearrange("(kt p) n -> p kt n", p=128))
                wff2 = wp.tile([128, 16, D], W_DT, tag="w")
                nc.sync.dma_start(wff2[:], w_ff2[l].rearrange("(kt p) n -> p kt n", p=128))

                # RoPE on hT -> hrT (pairs are (j, j+32) blocks within each head)
                hrT = ap.tile([128, 4, T], bf16, tag="actb")
                t2 = ap.tile([128, 4, T], f32, tag="ropesc", bufs=2)
                t1r = ap.tile([128, 4, T], f32, tag="ropesc", bufs=2)
                for m in range(4):
                    swp_ps = pmm.tile([128, T], f32, tag="mm")
                    nc.tensor.matmul(swp_ps[:], pswap[:], hT[:, m],
                                     start=True, stop=True)
                    nc.vector.tensor_mul(t2[:, m], hT[:, m], ct[:, m])
                    nc.vector.tensor_mul(t1r[:, m], swp_ps[:], sts[:, m])
                    nc.vector.tensor_tensor(hrT[:, m], t2[:, m], t1r[:, m], ALU.add)

                # q/k projections (Form T: out [Do,T])
                qT = ap.tile([128, 4, T], bf16, tag="actb")
                kT = ap.tile([128, 4, T], bf16, tag="actb")
                for qk, dst in ((0, qT), (1, kT)):
                    for m in range(4):
                        ps = pmm.tile([128, T], f32, tag="mm")
                        for k in range(4):
                            nc.tensor.matmul(
                                ps[:], wqkv[:, k, qk * D + m * 128: qk * D + (m + 1) * 128],
                                hrT[:, k], start=(k == 0), stop=(k == 3))
                        nc.scalar.activation(dst[:, m], ps[:], AT.Identity,
                                             bias=qkb[:, m, 2 * l + qk: 2 * l + qk + 1],
                                             scale=1.0)

                # v projection (Form N: out [T,D]); bias folded into out_b on host
                v = vp.tile([128, 2, D], W_DT, tag="v")
                for m2 in range(2):
                    for dh in range(2):
                        ps = pmm.tile([128, T], f32, tag="mm")
                        for k in range(4):
                            nc.tensor.matmul(
                                ps[:], hT[:, k, m2 * 128:(m2 + 1) * 128],
                                wqkv[:, k, 2 * D + dh * 256: 2 * D + (dh + 1) * 256],
                                start=(k == 0), stop=(k == 3))
                        nc.vector.tensor_copy(v[:, m2, dh * 256:(dh + 1) * 256], ps[:])

                # attention; sc phase of group kt overlaps ctx phase of kt-1
                ctxT = ap.tile([128, 4, T], bf16, tag="actb")
                atTs = {}

                def sc_phase(kt):
                    sc_t = {}
                    for half in range(2):
                        fr = slice(half * 128, (half + 1) * 128)
                        for hh in range(2):
                            sc = patt.tile([128, 128], f32, tag="sc",
                                           name=f"sc{kt}_{hh}_{half}")
                            nc.tensor.matmul(sc[:], mask8[:], idw8[:],
                                             start=True, stop=False, perf_mode=DRM)
                            sc_t[(hh, half)] = sc
                        for hh in range(2):  # adjacent for row-group packing
                            pb = 64 * hh
                            nc.tensor.matmul(sc_t[(hh, half)][:],
                                             qT[pb:pb + 64, kt, fr],
                                             kT[pb:pb + 64, kt, fr],
                                             start=False, stop=True)
                    for hh in range(2):
                        attn = atp.tile([128, 256], bf16, tag="atb")
                        for half in range(2):
                            att = atp.tile([128, 128], f32, tag="at")
                            rsum = stp.tile([128, 1], f32, tag="rs")
                            nc.scalar.activation(att[:], sc_t[(hh, half)][:], AT.Exp,
                                                 accum_out=rsum[:])
                            rinv = stp.tile([128, 1], f32, tag="rs")
                            nc.vector.reciprocal(rinv[:], rsum[:])
                            nc.vector.tensor_scalar_mul(attn[:, half * 128:(half + 1) * 128],
                                                        att[:], rinv[:])
                        atT = atp.tile([128, 256], bf16, tag="atb")
                        nc.vector.transpose(atT[:], attn[:])
                        atTs[(kt, hh)] = atT

                def ctx_phase(kt):
                    cps = pmm.tile([128, T], f32, tag="mm")
                    for half in range(2):
                        fr = slice(half * 128, (half + 1) * 128)
                        for hh in range(2):  # adjacent for col-group packing
                            h = 2 * kt + hh
                            pb = 64 * hh
                            nc.tensor.matmul(cps[pb:pb + 64, fr],
                                             v[:, half, h * 64:(h + 1) * 64],
                                             atTs[(kt, hh)][:, fr], start=True, stop=True)
                    nc.vector.tensor_copy(ctxT[:, kt, :], cps[:])

                for kt in range(4):
                    sc_phase(kt)
                    if kt >= 1:
                        ctx_phase(kt - 1)
                ctx_phase(3)

                # out projection + residual + ln1 stats inline
                h1pre = ap.tile([128, 4, T], bf16, tag="actb")
                sa4 = ap.tile([128, 4, T], f32, tag="resc", bufs=2)
                sum1 = pmm.tile([1, T], f32, tag="mm")
                sqs1 = pmm.tile([1, T], f32, tag="mm")
                sq1 = ap.tile([128, 4, T], bf16, tag="sqt", bufs=2)
                for m in range(4):
                    ps = pmm.tile([128, T], f32, tag="mm")
                    for k in range(4):
                        nc.tensor.matmul(ps[:], wout[:, k, m * 128:(m + 1) * 128],
                                         ctxT[:, k], start=(k == 0), stop=(k == 3))
                    nc.scalar.activation(sa4[:, m], ps[:], AT.Identity,
                                         bias=obt[:, m, l:l + 1], scale=1.0)
                    nc.vector.tensor_tensor(h1pre[:, m], sa4[:, m], hT[:, m], ALU.add)
                for m in range(4):
                    ln_stats(h1pre, m, sum1, sqs1, sq1)

                h1T = ap.tile([128, 4, T], bf16, tag="actb")
                ln_apply(h1pre, 4 * l, h1T, sum1, sqs1)

                # FFN
                gl = glp.tile([128, 16, T], W_DT, tag="gl")
                for ft in range(16):
                    ps = pmm.tile([128, T], f32, tag="mm")
                    for k in range(4):
                        nc.tensor.matmul(ps[:], wff1[:, k, ft * 128:(ft + 1) * 128],
                                         h1T[:, k], start=(k == 0), stop=(k == 3))
                    nc.scalar.activation(gl[:, ft], ps[:], AT.Gelu,
                                         bias=ff1b[:, ft, l:l + 1], scale=1.0)
                h2pre = ap.tile([128, 4, T], bf16, tag="actb")
                ff4 = ap.tile([128, 4, T], f32, tag="resc", bufs=2)
                sum2 = pmm.tile([1, T], f32, tag="mm")
                sqs2 = pmm.tile([1, T], f32, tag="mm")
                sq2 = ap.tile([128, 4, T], bf16, tag="sqt", bufs=2)
                for m in range(4):
                    ps = pmm.tile([128, T], f32, tag="mm")
                    for k in range(16):
                        nc.tensor.matmul(ps[:], wff2[:, k, m * 128:(m + 1) * 128],
                                         gl[:, k], start=(k == 0), stop=(k == 15))
                    nc.scalar.activation(ff4[:, m], ps[:], AT.Identity,
                                         bias=ff2b[:, m, l:l + 1], scale=1.0)
                    nc.vector.tensor_tensor(h2pre[:, m], ff4[:, m], h1T[:, m], ALU.add)
                for m in range(4):
                    ln_stats(h2pre, m, sum2, sqs2, sq2)

                hT = ap.tile([128, 4, T], bf16, tag="actb")
                ln_apply(h2pre, 4 * l + 2, hT, sum2, sqs2)

        # ---- output projection: out[T, IN] = h @ outp_w.T ------------------
        NCH = 12
        CW = IN // NCH            # 768 columns per chunk
        with tc.tile_pool(name="op_sb", bufs=4) as op:
            for ncr in range(NCH):
                wc = op.tile([128, 4, CW], IO_DT, tag="wco")
                nc.sync.dma_start(
                    wc[:], outp_wT.rearrange("(kt p) n -> p kt n", p=128)
                    [:, :, ncr * CW:(ncr + 1) * CW])
                for m2 in range(2):
                    pss = [pmm.tile([128, T], f32, tag="mm", name=f"ops{ncr}_{m2}_{nn}")
                           for nn in range(3)]
                    for k in range(4):
                        for nn in range(3):
                            nc.tensor.matmul(pss[nn][:],
                                             hT[:, k, m2 * 128:(m2 + 1) * 128],
                                             wc[:, k, nn * 256:(nn + 1) * 256],
                                             start=(k == 0), stop=(k == 3))
                    for nn in range(3):
                        osb = op.tile([128, 256], bf16, tag="osb")
                        nc.vector.tensor_copy(osb[:], pss[nn][:])
                        nc.sync.dma_start(
                            out_d[m2 * 128:(m2 + 1) * 128,
                                  ncr * CW + nn * 256: ncr * CW + (nn + 1) * 256],
                            osb[:])

    _split_multi_waits(nc)
    return nc


# ---------------------------------------------------------------------------
# host wrapper
# ---------------------------------------------------------------------------

def _prepare(inputs, nlayers):
    perm = _perm_src()
    inp_w = np.asarray(inputs["inp_w"], np.float32)
    inp_b = np.asarray(inputs["inp_b"], np.float32)
    in_proj_w = np.asarray(inputs["in_proj_w"], np.float32)
    in_proj_b = np.asarray(inputs["in_proj_b"], np.float32)
    out_w = np.asarray(inputs["out_w"], np.float32)
    out_b = np.asarray(inputs["out_b"], np.float32)
    ln1_w = np.asarray(inputs["ln1_w"], np.float32)
    ln1_b = np.asarray(inputs["ln1_b"], np.float32)
    ln2_w = np.asarray(inputs["ln2_w"], np.float32)
    ln2_b = np.asarray(inputs["ln2_b"], np.float32)
    ff1_w = np.asarray(inputs["ff1_w"], np.float32)
    ff1_b = np.asarray(inputs["ff1_b"], np.float32)
    ff2_w = np.asarray(inputs["ff2_w"], np.float32)
    ff2_b = np.asarray(inputs["ff2_b"], np.float32)
    outp_w = np.asarray(inputs["outp_w"], np.float32)

    common = {}
    common["inp_wT"] = _cast(inp_w[perm, :].T, IO_DT)
    common["outp_wT"] = _cast(outp_w[:, perm].T, IO_DT)
    qkb = np.zeros((D, 2 * nlayers), np.float32)
    obt = np.zeros((D, nlayers), np.float32)
    ff1bt = np.zeros((FF, nlayers), np.float32)
    ff2bt = np.zeros((D, nlayers), np.float32)
    lnc = np.zeros((D, 4 * nlayers), np.float32)
    for l in range(nlayers):
        wq = in_proj_w[l, :D][perm][:, perm] / np.sqrt(HD)
        wk = in_proj_w[l, D:2 * D][perm][:, perm]
        wv = in_proj_w[l, 2 * D:][perm][:, perm]
        common[f"w_qkv_{l}"] = _cast(np.concatenate([wq.T, wk.T, wv.T], axis=1), W_DT)
        common[f"w_out_{l}"] = _cast(out_w[l][perm][:, perm].T, W_DT)
        common[f"w_ff1_{l}"] = _cast(ff1_w[l][:, perm].T, W_DT)
        common[f"w_ff2_{l}"] = _cast(ff2_w[l][perm, :].T, W_DT)
        qkb[:, 2 * l] = in_proj_b[l, :D][perm] / np.sqrt(HD)
        qkb[:, 2 * l + 1] = in_proj_b[l, D:2 * D][perm]
        # v bias folded through attention (softmax rows sum to 1): obt = out_b + out_w @ bv
        bv = in_proj_b[l, 2 * D:]
        obt[:, l] = (out_b[l] + out_w[l] @ bv)[perm]
        ff1bt[:, l] = ff1_b[l]
        ff2bt[:, l] = ff2_b[l][perm]
        lnc[:, 4 * l] = ln1_w[l][perm] * D
        lnc[:, 4 * l + 1] = ln1_b[l][perm]
        lnc[:, 4 * l + 2] = ln2_w[l][perm] * D
        lnc[:, 4 * l + 3] = ln2_b[l][perm]
    common["qkb"] = qkb
    common["obt"] = obt
    common["ff1bt"] = ff1bt
    common["ff2bt"] = ff2bt
    common["lnc"] = lnc
    common["inpbt"] = inp_b[perm].reshape(D, 1).astype(np.float32)

    # rope tables in permuted space: partition d = h*64 + j
    theta = 1.0 / (10000.0 ** (np.arange(0, HD, 2, dtype=np.float32) / HD))  # [32]
    pos = np.arange(S, dtype=np.float32)
    ang = pos[:, None] * theta[None, :]                  # [S, 32]
    cos_t = np.cos(ang).astype(np.float32)
    sin_t = np.sin(ang).astype(np.float32)
    ct = np.zeros((D, T), np.float32)
    sts = np.zeros((D, T), np.float32)
    s_of_t = np.tile(np.arange(S), BSH)                  # position of each token
    for h in range(NH):
        for j in range(HD // 2):
            ct[h * HD + j, :] = cos_t[s_of_t, j]
            ct[h * HD + HD // 2 + j, :] = cos_t[s_of_t, j]
            sts[h * HD + j, :] = -sin_t[s_of_t, j]
            sts[h * HD + HD // 2 + j, :] = sin_t[s_of_t, j]
    common["ct"] = _cast(ct, bf16)
    common["sts"] = _cast(sts, bf16)

    # additive causal+block mask as fp8-e5m2 DoubleRow pair (subtile 1 is zero)
    mask = np.full((128, 128), NEG, np.float32)
    for blk in range(4):
        for i in range(S):
            mask[blk * S + i, blk * S: blk * S + i + 1] = 0.0
    m8 = np.zeros((128, 2, 128), ml_dtypes.float8_e5m2)
    m8[:, 0, :] = mask.T.astype(ml_dtypes.float8_e5m2)
    common["mask8"] = m8
    i8 = np.zeros((128, 2, 128), ml_dtypes.float8_e5m2)
    i8[:, 0, :] = np.eye(128, dtype=np.float32).astype(ml_dtypes.float8_e5m2)
    common["idw8"] = i8
    common["onesb"] = _cast(np.ones((128, 1), np.float32), bf16)
    common["onesr"] = _round_f32r(np.ones((1, 128), np.float32))
    psw = np.zeros((128, 128), np.float32)
    for h2 in range(2):
        b0 = 64 * h2
        for j in range(32):
            psw[b0 + 32 + j, b0 + j] = 1.0      # lhsT[k, m]: out[m] sums in[k]
            psw[b0 + j, b0 + 32 + j] = 1.0
    common["pswap"] = _cast(psw, bf16)
    return common


def kernel(**inputs):
    nlayers = _CACHE.setdefault("nlayers", L)
    x = np.asarray(inputs["x"], np.float32)
    if "bass" not in _CACHE:
        _CACHE["bass"] = _build(nlayers)
    nc = _CACHE["bass"]
    common = _prepare(inputs, nlayers)
    in_maps = []
    for c in range(NCORES):
        m = dict(common)
        xc = x[c * BSH:(c + 1) * BSH].reshape(T, IN)
        m["xT"] = _cast(xc.T, IO_DT)
        in_maps.append(m)
    res = run_bass_kernel_spmd(nc, in_maps, core_ids=list(range(NCORES)))
    _CACHE["res"] = res
    outp_b = np.asarray(inputs["outp_b"], np.float32)
    outs = [np.asarray(res.results[c]["out"], np.float32) + outp_b[None, :]
            for c in range(NCORES)]
    full = np.concatenate(outs, axis=0).reshape(B, S, LD, Hh, Ww)
    return full.astype(np.float32)
